# revision 1
# baseline (speedup 1.0000x reference)
"""Trainium2 Bass kernel for GraphTransitionModel (GNN message passing).

Model (per batch element b, N=256 nodes):
  x[i]   = (obs[b,i], i/N)                              node features, 2-dim
  h1     = relu(W0a^T x_i + W0b^T x_j + a*w4 + b0)      messenger layer 1, 64
  h2     = relu(W1^T h1 + b1)                           64
  h3     = relu(W2^T h2 + b2)                           64
  m(i,j) = w3 . h3 + b3                                 scalar
  msg[i] = sum_j m(i,j) = w3 . (sum_j h3) + N*b3
  u      = MLP_updater([x_i, msg[i]])  (3->64->64->64->1)
  out[b,i] = u

Strategy: pure data parallel, 4 batch elements per core x 8 cores.
On-chip layout: features on partitions, pairs on the free dim.
Two i-rows (i and i+128) are stacked into 128 partitions; the 64x64
layers run as 128x128 block-diagonal matmuls.  The final w3-dot plus
sum over j is folded into a per-i free-dim accumulation (accum_out).

Sync-wait budget: Trainium matmul (S3_LW) carries a single sync-wait
slot, so every matmul's operands must be reachable through one
semaphore: constants arrive via ONE packed DMA, a barrier + dummy PE
matmul absorbs that dep, and multi-writer tiles (qb, pb, s2) are
fenced through single DVE copies.
"""

import os
import sys
import numpy as np

sys.path.insert(0, "/opt/trn_rl_repo")

B, N, MID = 32, 256, 64
NCORES = 8
BPC = B // NCORES  # batches per core = 4
HALF = N // 2  # 128 stacked tiles per batch

# wpack column layout
C_W1BD = 0
C_W2BD = 128
C_UW1 = 256
C_UW2 = 320
C_W0A = 384
C_W0B = 448
C_UW0 = 512
C_W3S = 576
C_B1S = 578
C_B2S = 579
C_UW3 = 580
C_UB0 = 581
C_UB1 = 582
C_UB2 = 583
C_SCAL = 584
C_TOT = 586


def _build_bass():
    import concourse.bass as bass
    import concourse.bacc as bacc
    import concourse.tile as tile
    from concourse import mybir

    f32 = mybir.dt.float32
    AF = mybir.ActivationFunctionType
    ALU = mybir.AluOpType

    nc = bacc.Bacc("TRN2", target_bir_lowering=False, num_devices=NCORES)

    wpack_d = nc.declare_dram_parameter("wpack", [128, C_TOT], f32, isOutput=False)
    xT_d = nc.declare_dram_parameter("xT", [BPC, 2, N], f32, isOutput=False)
    ab0_d = nc.declare_dram_parameter("ab0", [BPC, MID, 1], f32, isOutput=False)
    out_d = nc.declare_dram_parameter("out", [BPC, N], f32, isOutput=True)

    with tile.TileContext(nc) as tc:
        with (
            tc.tile_pool(name="consts", bufs=1) as consts,
            tc.tile_pool(name="perb", bufs=2) as perb,
            tc.tile_pool(name="work", bufs=3) as work,
            tc.tile_pool(name="ps_main", bufs=3, space="PSUM") as ps_main,
            tc.tile_pool(name="ps_main2", bufs=3, space="PSUM") as ps_main2,
            tc.tile_pool(name="ps_small", bufs=1, space="PSUM") as ps_small,
            tc.tile_pool(name="ps_warm", bufs=1, space="PSUM") as ps_warm,
        ):
            wp = consts.tile([128, C_TOT], f32, tag="wpack")
            nc.sync.dma_start(out=wp[:], in_=wpack_d[:])
            w1bd = wp[:, C_W1BD : C_W1BD + 128]
            w2bd = wp[:, C_W2BD : C_W2BD + 128]
            uw1 = wp[0:MID, C_UW1 : C_UW1 + MID]
            uw2 = wp[0:MID, C_UW2 : C_UW2 + MID]
            w0a = wp[0:2, C_W0A : C_W0A + MID]
            w0b = wp[0:2, C_W0B : C_W0B + MID]
            uw0 = wp[0:3, C_UW0 : C_UW0 + MID]
            w3s = wp[:, C_W3S : C_W3S + 2]
            b1s = wp[:, C_B1S : C_B1S + 1]
            b2s = wp[:, C_B2S : C_B2S + 1]
            uw3 = wp[0:MID, C_UW3 : C_UW3 + 1]
            ub0 = wp[0:MID, C_UB0 : C_UB0 + 1]
            ub1 = wp[0:MID, C_UB1 : C_UB1 + 1]
            ub2 = wp[0:MID, C_UB2 : C_UB2 + 1]
            scal = wp[:, C_SCAL : C_SCAL + 2]

            # Dummy PE matmul absorbs the wpack-DMA wait so later matmuls
            # (single sync-wait slot) only wait on their RAW producer engine.
            psw = ps_warm.tile([1, 1], f32, tag="warm")
            nc.tensor.matmul(psw[:], w1bd[:, 0:1], w1bd[:, 0:1], start=True, stop=True)

            for b in range(BPC):
                # ---- per-batch setup ----
                uin = perb.tile([3, N], f32, tag="uin")
                nc.sync.dma_start(out=uin[0:2, :], in_=xT_d[b])
                ab0s = perb.tile([128, 1], f32, tag="ab0s")
                src = ab0_d[b]
                ab0_bcast = bass.AP(
                    tensor=src.tensor,
                    offset=src.offset,
                    ap=[[0, 2]] + list(src.ap),
                )
                nc.sync.dma_start(out=ab0s[:], in_=ab0_bcast)

                psP = ps_small.tile([MID, N], f32, tag="pss")
                nc.tensor.matmul(psP[:], w0a, uin[0:2, :], start=True, stop=True)
                p1 = perb.tile([MID, N], f32, tag="p1")
                nc.scalar.copy(p1[:], psP[:])

                psQ = ps_small.tile([MID, N], f32, tag="pss")
                nc.tensor.matmul(psQ[:], w0b, uin[0:2, :], start=True, stop=True)
                qb = perb.tile([128, N], f32, tag="qb")
                nc.scalar.activation(qb[0:MID, :], psQ[:], AF.Identity, bias=ab0s[0:MID, :])
                nc.sync.dma_start(out=qb[MID:128, :], in_=qb[0:MID, :])

                pb = perb.tile([128, HALF], f32, tag="pb")
                nc.sync.dma_start(out=pb[0:MID, :], in_=p1[:, 0:HALF])
                nc.sync.dma_start(out=pb[MID:128, :], in_=p1[:, HALF:N])

                # DVE fences: h1's tensor_scalar then depends only on DVE
                if b == 0:
                    zeros = consts.tile([128, N], f32, tag="zeros")
                    nc.vector.memset(zeros[:], 0.0)
                qb2 = perb.tile([128, N], f32, tag="qb2")
                nc.vector.tensor_copy(qb2[:], qb[:])
                pb2 = perb.tile([128, HALF], f32, tag="pb2")
                nc.vector.tensor_copy(pb2[:], pb[:])

                # S2[c, t] accumulates sum_j h3 for i=t (upper) / i=t+HALF (lower)
                # ACT and DVE accumulate into separate tiles to avoid any
                # false cross-engine WAW serialization on a shared tile
                s2 = perb.tile([128, HALF], f32, tag="s2")
                s2a = perb.tile([128, (HALF + 2) // 3], f32, tag="s2a")

                # ---- main pair loop ----
                for t in range(HALF):
                    h1 = work.tile([128, N], f32, tag="h1")
                    nc.vector.tensor_scalar(
                        h1[:], qb2[:], pb2[:, t : t + 1], 0.0, ALU.add, ALU.max
                    )
                    ps1 = ps_main.tile([128, N], f32, tag="ps1")
                    nc.tensor.matmul(ps1[:], w1bd, h1[:], start=True, stop=True)
                    h2 = work.tile([128, N], f32, tag="h2")
                    nc.scalar.activation(h2[:], ps1[:], AF.Relu, bias=b1s)
                    ps2 = ps_main2.tile([128, N], f32, tag="ps2")
                    nc.tensor.matmul(ps2[:], w2bd, h2[:], start=True, stop=True)
                    h3 = work.tile([128, N], f32, tag="h3")
                    if t % 3 != 0:
                        # relu+sum-accum on DVE: (ps2 + b2s) max 0; accum is
                        # hardwired to sum for scalar_tensor_tensor
                        nc.vector.scalar_tensor_tensor(
                            h3[:], ps2[:], b2s, zeros[:], ALU.add, ALU.max,
                            accum_out=s2[:, t : t + 1],
                        )
                    else:
                        nc.scalar.activation(
                            h3[:], ps2[:], AF.Relu, bias=b2s,
                            accum_out=s2a[:, t // 3 : t // 3 + 1],
                        )

                # ---- msg = w3s^T @ S2  -> [2, HALF] ----
                s2f = perb.tile([128, HALF], f32, tag="s2f")
                nc.vector.tensor_copy(s2f[:], s2[:])
                sel = bass.AP(tensor=s2f.tensor, offset=s2f.offset,
                              ap=[s2f.ap[0], [3, (HALF + 2) // 3]])
                nc.vector.tensor_copy(sel, s2a[:])
                psm = ps_small.tile([2, HALF], f32, tag="pss")
                nc.tensor.matmul(psm[:], w3s, s2f[:], start=True, stop=True)
                msg2 = perb.tile([2, HALF], f32, tag="msg2")
                nc.scalar.activation(msg2[:], psm[:], AF.Identity, bias=scal[0:2, 0:1])
                # flatten [2, HALF] -> row 2 of uin [1, N]
                nc.sync.dma_start(out=uin[2:3, :], in_=msg2[:])

                # ---- updater MLP ----
                psu1 = ps_small.tile([MID, N], f32, tag="pss")
                nc.tensor.matmul(psu1[:], uw0, uin[:], start=True, stop=True)
                t1 = perb.tile([MID, N], f32, tag="t1")
                nc.scalar.activation(t1[:], psu1[:], AF.Relu, bias=ub0)
                psu2 = ps_small.tile([MID, N], f32, tag="pss")
                nc.tensor.matmul(psu2[:], uw1, t1[:], start=True, stop=True)
                t2 = perb.tile([MID, N], f32, tag="t2")
                nc.scalar.activation(t2[:], psu2[:], AF.Relu, bias=ub1)
                psu3 = ps_small.tile([MID, N], f32, tag="pss")
                nc.tensor.matmul(psu3[:], uw2, t2[:], start=True, stop=True)
                t3 = perb.tile([MID, N], f32, tag="t3")
                nc.scalar.activation(t3[:], psu3[:], AF.Relu, bias=ub2)
                pso = ps_small.tile([1, N], f32, tag="pss")
                nc.tensor.matmul(pso[:], uw3, t3[:], start=True, stop=True)
                orow = perb.tile([1, N], f32, tag="orow")
                nc.scalar.activation(orow[:], pso[:], AF.Identity, bias=scal[0:1, 1:2])
                nc.sync.dma_start(out=out_d[b], in_=orow[:])

    nc.compile()
    return nc


def _host_inputs(inputs):
    g = lambda k: np.asarray(inputs[k], np.float32)
    obs, action = g("obs"), g("action")
    m_w0, m_b0, m_w1, m_b1 = g("m_w0"), g("m_b0"), g("m_w1"), g("m_b1")
    m_w2, m_b2, m_w3, m_b3 = g("m_w2"), g("m_b2"), g("m_w3"), g("m_b3")
    u_w0, u_b0, u_w1, u_b1 = g("u_w0"), g("u_b0"), g("u_w1"), g("u_b1")
    u_w2, u_b2, u_w3, u_b3 = g("u_w2"), g("u_b2"), g("u_w3"), g("u_b3")

    coor = np.arange(N, dtype=np.float32) / N
    xT = np.stack([obs, np.broadcast_to(coor, obs.shape)], axis=1)  # [B, 2, N]
    ab0 = (action[:, None] * m_w0[4] + m_b0).astype(np.float32)[..., None]

    wpack = np.zeros((128, C_TOT), np.float32)
    wpack[:MID, C_W1BD : C_W1BD + MID] = m_w1
    wpack[MID:, C_W1BD + MID : C_W1BD + 128] = m_w1
    wpack[:MID, C_W2BD : C_W2BD + MID] = m_w2
    wpack[MID:, C_W2BD + MID : C_W2BD + 128] = m_w2
    wpack[:MID, C_UW1 : C_UW1 + MID] = u_w1
    wpack[:MID, C_UW2 : C_UW2 + MID] = u_w2
    wpack[0:2, C_W0A : C_W0A + MID] = m_w0[0:2]
    wpack[0:2, C_W0B : C_W0B + MID] = m_w0[2:4]
    wpack[0:3, C_UW0 : C_UW0 + MID] = u_w0
    wpack[:MID, C_W3S] = m_w3[:, 0]
    wpack[MID:, C_W3S + 1] = m_w3[:, 0]
    wpack[:MID, C_B1S] = m_b1
    wpack[MID:, C_B1S] = m_b1
    wpack[:MID, C_B2S] = m_b2
    wpack[MID:, C_B2S] = m_b2
    wpack[:MID, C_UW3] = u_w3[:, 0]
    wpack[:MID, C_UB0] = u_b0
    wpack[:MID, C_UB1] = u_b1
    wpack[:MID, C_UB2] = u_b2
    wpack[0:2, C_SCAL] = N * float(m_b3[0])
    wpack[0:2, C_SCAL + 1] = float(u_b3[0])

    in_maps = []
    for c in range(NCORES):
        sl = slice(c * BPC, (c + 1) * BPC)
        in_maps.append(
            dict(
                wpack=wpack,
                xT=np.ascontiguousarray(xT[sl]),
                ab0=np.ascontiguousarray(ab0[sl]),
            )
        )
    return in_maps


def kernel(**inputs) -> np.ndarray:
    in_maps = _host_inputs(inputs)

    from concourse.bass_utils import run_bass_kernel_spmd

    nc = _build_bass()
    res = run_bass_kernel_spmd(
        nc, in_maps, core_ids=list(range(NCORES)),
        trace=bool(int(os.environ.get("KERNEL_TRACE", "0"))),
    )
    out = np.concatenate([r["out"] for r in res.results], axis=0)  # [B, N]
    if res.exec_time_ns is not None:
        print(f"HW exec time: {res.exec_time_ns} ns")
        print(f"mean exec time: {res.mean_exec_time_ns} ns")
    return out.astype(np.float32)


if __name__ == "__main__":
    nc = _build_bass()
    print("bass build OK")



# revision 12
# speedup vs baseline: 1.3728x; 1.3728x over previous
"""Trainium2 Bass kernel for GraphTransitionModel (GNN message passing).

Model (per batch element b, N=256 nodes):
  x[i]   = (obs[b,i], i/N)                              node features, 2-dim
  h1     = relu(W0a^T x_i + W0b^T x_j + a*w4 + b0)      messenger layer 1, 64
  h2     = relu(W1^T h1 + b1)                           64
  h3     = relu(W2^T h2 + b2)                           64
  m(i,j) = w3 . h3 + b3                                 scalar
  msg[i] = sum_j m(i,j) = w3 . (sum_j h3) + N*b3
  u      = MLP_updater([x_i, msg[i]])  (3->64->64->64->1)
  out[b,i] = u

Strategy: pure data parallel, 4 batch elements per core x 8 cores.
Features on partitions, pairs on the free dim; two i-rows (i and
i+128) stacked into 128 partitions so the 64x64 layers run as 128x128
block-diagonal matmuls.

v2: all pair-loop matmuls in bf16 (fp32 matmul = 4 cycles/row on PE),
t-loop processed in chunks of CH=4 so h2 is evacuated as one chunked
ACT op per 4 t's; h3 relu+j-sum split DVE-majority/ACT; h1 built on
DVE in bf16 (2x mode).  PSUM: psA/psB pools 2 bufs x 2 banks each.
"""

import os
import sys
import numpy as np

sys.path.insert(0, "/opt/trn_rl_repo")

B, N, MID = 32, 256, 64
NCORES = 8
BPC = B // NCORES  # batches per core = 4
HALF = N // 2  # 128 stacked tiles per batch
CH = 4  # t's per chunk
NCHUNK = HALF // CH

# wpack column layout (fp32 elements)
C_W1BD = 0
C_W2BD = 128
C_UW1 = 256
C_UW2 = 320
C_W0A = 384
C_W0B = 448
C_UW0 = 512
C_W3S = 576
C_B1S = 578
C_B2S = 579
C_UW3 = 580
C_UB0 = 581
C_UB1 = 582
C_UB2 = 583
C_SCAL = 584
C_TOT = 586


def _build_bass():
    import concourse.bass as bass
    import concourse.bacc as bacc
    import concourse.tile as tile
    from concourse import mybir

    f32 = mybir.dt.float32
    bf16 = mybir.dt.bfloat16
    AF = mybir.ActivationFunctionType
    ALU = mybir.AluOpType

    nc = bacc.Bacc("TRN2", target_bir_lowering=False, num_devices=NCORES)

    wpack_d = nc.declare_dram_parameter("wpack", [128, C_TOT], f32, isOutput=False)
    xT_d = nc.declare_dram_parameter("xT", [BPC, 2, N], f32, isOutput=False)
    ab0_d = nc.declare_dram_parameter("ab0", [BPC, MID, 1], f32, isOutput=False)
    out_d = nc.declare_dram_parameter("out", [BPC, N], f32, isOutput=True)

    with tile.TileContext(nc) as tc:
        with (
            tc.tile_pool(name="consts", bufs=1) as consts,
            tc.tile_pool(name="perb", bufs=2) as perb,
            tc.tile_pool(name="h1p", bufs=2) as h1p,
            tc.tile_pool(name="h2p", bufs=2) as h2p,
            tc.tile_pool(name="work", bufs=4) as work,
            tc.tile_pool(name="ps_a", bufs=2, space="PSUM") as ps_a,
            tc.tile_pool(name="ps_b", bufs=2, space="PSUM") as ps_b,
            tc.tile_pool(name="ps_s", bufs=2, space="PSUM") as ps_s,
        ):
            wp = consts.tile([128, C_TOT], f32, tag="wpack")
            nc.sync.dma_start(out=wp[:], in_=wpack_d[:])
            w1bd = wp[:, C_W1BD : C_W1BD + 128]
            w2bd = wp[:, C_W2BD : C_W2BD + 128]
            uw1 = wp[0:MID, C_UW1 : C_UW1 + MID]
            uw2 = wp[0:MID, C_UW2 : C_UW2 + MID]
            w0a = wp[0:2, C_W0A : C_W0A + MID]
            w0b = wp[0:2, C_W0B : C_W0B + MID]
            uw0 = wp[0:3, C_UW0 : C_UW0 + MID]
            w3s = wp[:, C_W3S : C_W3S + 2]
            b1s = wp[:, C_B1S : C_B1S + 1]
            b2s = wp[:, C_B2S : C_B2S + 1]
            uw3 = wp[0:MID, C_UW3 : C_UW3 + 1]
            ub0 = wp[0:MID, C_UB0 : C_UB0 + 1]
            ub1 = wp[0:MID, C_UB1 : C_UB1 + 1]
            ub2 = wp[0:MID, C_UB2 : C_UB2 + 1]
            scal = wp[:, C_SCAL : C_SCAL + 2]

            # Dummy PE matmul absorbs the wpack-DMA wait so later matmuls
            # (single sync-wait slot) only wait on their RAW producer engine.
            psw = ps_s.tile([1, 1], f32, tag="pss")
            nc.tensor.matmul(psw[:], w1bd[:, 0:1], w1bd[:, 0:1], start=True, stop=True)

            # bf16 copies of the pair-loop weights.  w1b on DVE so the L2
            # matmul's deps are DVE-only; w2b on ACT so L3's are ACT-only.
            w1b = consts.tile([128, 128], bf16, tag="w1b")
            nc.vector.tensor_copy(w1b[:], w1bd)
            w2b = consts.tile([128, 128], bf16, tag="w2b")
            nc.scalar.copy(w2b[:], w2bd)

            zeros = consts.tile([128, N], f32, tag="zeros")
            nc.vector.memset(zeros[:], 0.0)

            for b in range(BPC):
                # ---- per-batch setup ----
                uin = perb.tile([3, N], f32, tag="uin")
                nc.sync.dma_start(out=uin[0:2, :], in_=xT_d[b])
                ab0s = perb.tile([128, 1], f32, tag="ab0s")
                src = ab0_d[b]
                ab0_bcast = bass.AP(
                    tensor=src.tensor,
                    offset=src.offset,
                    ap=[[0, 2]] + list(src.ap),
                )
                nc.sync.dma_start(out=ab0s[:], in_=ab0_bcast)

                psP = ps_s.tile([MID, N], f32, tag="pss")
                nc.tensor.matmul(psP[:], w0a, uin[0:2, :], start=True, stop=True)
                p1 = perb.tile([MID, N], f32, tag="p1")
                nc.scalar.copy(p1[:], psP[:])

                psQ = ps_s.tile([MID, N], f32, tag="pss")
                nc.tensor.matmul(psQ[:], w0b, uin[0:2, :], start=True, stop=True)
                qb = perb.tile([128, N], f32, tag="qb")
                nc.scalar.activation(qb[0:MID, :], psQ[:], AF.Identity, bias=ab0s[0:MID, :])
                nc.sync.dma_start(out=qb[MID:128, :], in_=qb[0:MID, :])

                pb = perb.tile([128, HALF], f32, tag="pb")
                nc.sync.dma_start(out=pb[0:MID, :], in_=p1[:, 0:HALF])
                nc.sync.dma_start(out=pb[MID:128, :], in_=p1[:, HALF:N])

                # DVE fences: single-producer (DVE) tiles for the pair loop
                qb2 = perb.tile([128, N], bf16, tag="qb2")
                nc.vector.tensor_copy(qb2[:], qb[:])
                pb2 = perb.tile([128, HALF], f32, tag="pb2")
                nc.vector.tensor_copy(pb2[:], pb[:])

                # S2[c, t] = sum_j h3 for i=t (upper) / i=t+HALF (lower)
                # DVE-accumulated columns in s2, ACT-accumulated in s2a
                s2 = perb.tile([128, HALF], f32, tag="s2")
                s2a = perb.tile([128, (HALF + 2) // 3], f32, tag="s2a")

                # ---- main pair loop, chunks of CH t's ----
                for c in range(NCHUNK):
                    t0 = c * CH
                    h1 = h1p.tile([128, CH * N], bf16, tag="h1")
                    for k in range(CH):
                        nc.vector.tensor_scalar(
                            h1[:, k * N : (k + 1) * N],
                            qb2[:],
                            pb2[:, t0 + k : t0 + k + 1],
                            0.0,
                            ALU.add,
                            ALU.max,
                        )
                    psA = ps_a.tile([128, CH * N], f32, tag="psA")
                    nc.tensor.matmul(
                        psA[:, 0:512], w1b[:], h1[:, 0:512], start=True, stop=True
                    )
                    nc.tensor.matmul(
                        psA[:, 512:1024], w1b[:], h1[:, 512:1024], start=True, stop=True
                    )
                    h2 = h2p.tile([128, CH * N], bf16, tag="h2")
                    nc.scalar.activation(h2[:], psA[:], AF.Relu, bias=b1s)
                    psB0 = ps_b.tile([128, 512], f32, tag="psB")
                    nc.tensor.matmul(
                        psB0[:], w2b[:], h2[:, 0:512], start=True, stop=True
                    )
                    psB1 = ps_b.tile([128, 512], f32, tag="psB")
                    nc.tensor.matmul(
                        psB1[:], w2b[:], h2[:, 512:1024], start=True, stop=True
                    )
                    for k in range(CH):
                        t = t0 + k
                        pbt = psB0 if k < 2 else psB1
                        sl = pbt[:, (k % 2) * N : (k % 2 + 1) * N]
                        h3 = work.tile([128, N], bf16, tag="h3")
                        if t % 3 != 0:
                            nc.vector.scalar_tensor_tensor(
                                h3[:], sl, b2s, zeros[:], ALU.add, ALU.max,
                                accum_out=s2[:, t : t + 1],
                            )
                        else:
                            nc.scalar.activation(
                                h3[:], sl, AF.Relu, bias=b2s,
                                accum_out=s2a[:, t // 3 : t // 3 + 1],
                            )

                # ---- msg = w3s^T @ S2  -> [2, HALF] ----
                s2f = perb.tile([128, HALF], f32, tag="s2f")
                nc.vector.tensor_copy(s2f[:], s2[:])
                sel = bass.AP(tensor=s2f.tensor, offset=s2f.offset,
                              ap=[s2f.ap[0], [3, (HALF + 2) // 3]])
                nc.vector.tensor_copy(sel, s2a[:])
                psm = ps_s.tile([2, HALF], f32, tag="pss")
                nc.tensor.matmul(psm[:], w3s, s2f[:], start=True, stop=True)
                msg2 = perb.tile([2, HALF], f32, tag="msg2")
                nc.scalar.activation(msg2[:], psm[:], AF.Identity, bias=scal[0:2, 0:1])
                # flatten [2, HALF] -> row 2 of uin [1, N]
                nc.sync.dma_start(out=uin[2:3, :], in_=msg2[:])

                # ---- updater MLP ----
                psu1 = ps_s.tile([MID, N], f32, tag="pss")
                nc.tensor.matmul(psu1[:], uw0, uin[:], start=True, stop=True)
                t1 = perb.tile([MID, N], f32, tag="t1")
                nc.scalar.activation(t1[:], psu1[:], AF.Relu, bias=ub0)
                psu2 = ps_s.tile([MID, N], f32, tag="pss")
                nc.tensor.matmul(psu2[:], uw1, t1[:], start=True, stop=True)
                t2 = perb.tile([MID, N], f32, tag="t2")
                nc.scalar.activation(t2[:], psu2[:], AF.Relu, bias=ub1)
                psu3 = ps_s.tile([MID, N], f32, tag="pss")
                nc.tensor.matmul(psu3[:], uw2, t2[:], start=True, stop=True)
                t3 = perb.tile([MID, N], f32, tag="t3")
                nc.scalar.activation(t3[:], psu3[:], AF.Relu, bias=ub2)
                pso = ps_s.tile([1, N], f32, tag="pss")
                nc.tensor.matmul(pso[:], uw3, t3[:], start=True, stop=True)
                orow = perb.tile([1, N], f32, tag="orow")
                nc.scalar.activation(orow[:], pso[:], AF.Identity, bias=scal[0:1, 1:2])
                nc.sync.dma_start(out=out_d[b], in_=orow[:])

    nc.compile()
    return nc


def _host_inputs(inputs):
    g = lambda k: np.asarray(inputs[k], np.float32)
    obs, action = g("obs"), g("action")
    m_w0, m_b0, m_w1, m_b1 = g("m_w0"), g("m_b0"), g("m_w1"), g("m_b1")
    m_w2, m_b2, m_w3, m_b3 = g("m_w2"), g("m_b2"), g("m_w3"), g("m_b3")
    u_w0, u_b0, u_w1, u_b1 = g("u_w0"), g("u_b0"), g("u_w1"), g("u_b1")
    u_w2, u_b2, u_w3, u_b3 = g("u_w2"), g("u_b2"), g("u_w3"), g("u_b3")

    coor = np.arange(N, dtype=np.float32) / N
    xT = np.stack([obs, np.broadcast_to(coor, obs.shape)], axis=1)  # [B, 2, N]
    ab0 = (action[:, None] * m_w0[4] + m_b0).astype(np.float32)[..., None]

    wpack = np.zeros((128, C_TOT), np.float32)
    wpack[:MID, C_W1BD : C_W1BD + MID] = m_w1
    wpack[MID:, C_W1BD + MID : C_W1BD + 128] = m_w1
    wpack[:MID, C_W2BD : C_W2BD + MID] = m_w2
    wpack[MID:, C_W2BD + MID : C_W2BD + 128] = m_w2
    wpack[:MID, C_UW1 : C_UW1 + MID] = u_w1
    wpack[:MID, C_UW2 : C_UW2 + MID] = u_w2
    wpack[0:2, C_W0A : C_W0A + MID] = m_w0[0:2]
    wpack[0:2, C_W0B : C_W0B + MID] = m_w0[2:4]
    wpack[0:3, C_UW0 : C_UW0 + MID] = u_w0
    wpack[:MID, C_W3S] = m_w3[:, 0]
    wpack[MID:, C_W3S + 1] = m_w3[:, 0]
    wpack[:MID, C_B1S] = m_b1
    wpack[MID:, C_B1S] = m_b1
    wpack[:MID, C_B2S] = m_b2
    wpack[MID:, C_B2S] = m_b2
    wpack[:MID, C_UW3] = u_w3[:, 0]
    wpack[:MID, C_UB0] = u_b0
    wpack[:MID, C_UB1] = u_b1
    wpack[:MID, C_UB2] = u_b2
    wpack[0:2, C_SCAL] = N * float(m_b3[0])
    wpack[0:2, C_SCAL + 1] = float(u_b3[0])

    in_maps = []
    for c in range(NCORES):
        sl = slice(c * BPC, (c + 1) * BPC)
        in_maps.append(
            dict(
                wpack=wpack,
                xT=np.ascontiguousarray(xT[sl]),
                ab0=np.ascontiguousarray(ab0[sl]),
            )
        )
    return in_maps


def kernel(**inputs) -> np.ndarray:
    in_maps = _host_inputs(inputs)

    from concourse.bass_utils import run_bass_kernel_spmd

    nc = _build_bass()
    res = run_bass_kernel_spmd(
        nc, in_maps, core_ids=list(range(NCORES)),
        trace=bool(int(os.environ.get("KERNEL_TRACE", "0"))),
    )
    out = np.concatenate([r["out"] for r in res.results], axis=0)  # [B, N]
    if res.exec_time_ns is not None:
        print(f"HW exec time: {res.exec_time_ns} ns")
        print(f"mean exec time: {res.mean_exec_time_ns} ns")
    return out.astype(np.float32)


if __name__ == "__main__":
    nc = _build_bass()
    print("bass build OK")
